# revision 15
# baseline (speedup 1.0000x reference)
"""Trainium2 Bass kernel: 3D BFP activation quantization (shared-exponent blocks
of blk=16 contiguous channels along C), data-parallel over batch N across 8
NeuronCores.

kernel(activations[8,64,32,64,64] f32, mantissa=7, blk=16) -> same-shape f32.

Math (exact fp32/int32 bit arithmetic; matches the jnp reference bit-for-bit):
  per block b, spatial s:  M = max_i |x[blk*b+i, s]|
  E  = exponent field of M;  quantum = 2^(E-127-(mant-1));  scale = 1/quantum
  y   = x * scale                                  # exact (pow2), |y| < 2^mant
  t   = min(y + 1.5*2^23, 1.5*2^23 + lim)          # RNE round to int + high clip
  a   = relu(t - (1.5*2^23 - lim))                 # low clip; a = clipped + lim
  out = (a - lim) * quantum                        # exact

Bit manipulation (int32, avoiding signed overflow and bitwise/arith mixing):
  V  = (Mbits | 0x807FFFFF) ^ -1   = 0x7F800000 - Ebits          (pure bitwise)
  Sbits = min(V, 0x73000000) + 0x02800000                        (scale, clamped
          so an all-zero block yields a finite scale; exact for any E >= 25)
  Ebits = Mbits & 0x7F800000
  Qbits = max(Ebits, 0x0C800000) - ((mant-1)<<23)                (quantum)

Layout: partition = 128 spatial positions, free = (C=64 channels, F spatial).
One 3D DMA per tile; block reduction over i done with a strided access pattern.
"""

import os
import sys

for _p in ("/opt/trn_rl_repo", "/root/.axon_site/_ro/trn_rl_repo"):
    if os.path.isdir(_p) and _p not in sys.path:
        sys.path.insert(0, _p)

import numpy as np

# ---- hardcoded problem geometry ----
N, C, D, H, W = 8, 64, 32, 64, 64
S = D * H * W                 # 131072 spatial per (n, c)
N_CORES = 8
PD = 128                      # SBUF partitions (spatial)

_BUILT = {}


def _build(mant: int, blk: int, F: int = 512, CT: int = 16, bufs: int = 3,
           mult_engine: str = "gpsimd", relu_engine: str = "scalar",
           store_engine: str = "scalar", mult_df: int = 0,
           C: int = C, S: int = S, mode: str = "full", repeat: int = 1):
    import concourse.bass as bass
    import concourse.bacc as bacc
    import concourse.mybir as mybir
    from concourse.tile import TileContext

    FP32 = mybir.dt.float32
    I32 = mybir.dt.int32
    Alu = mybir.AluOpType

    NBt = CT // blk           # channel blocks per tile
    NCC = C // CT             # channel chunks
    NTS = S // (PD * F)       # spatial chunks
    assert S % (PD * F) == 0 and C % CT == 0 and CT % blk == 0

    LIM = float(2 ** mant - 1)
    MAGIC = 1.5 * 2.0 ** 23
    EXP_OFF = (mant - 1) << 23

    nc = bacc.Bacc("TRN2", target_bir_lowering=False)
    x_d = nc.dram_tensor("x", [C, S], FP32, kind="ExternalInput")
    o_d = nc.dram_tensor("o", [C, S], FP32, kind="ExternalOutput")

    xr = x_d[:].rearrange("(cc ct) (ts sh f) -> ts cc sh ct f",
                          cc=NCC, ct=CT, ts=NTS, sh=PD, f=F)
    orr = o_d[:].rearrange("(cc ct) (ts sh f) -> ts cc sh ct f",
                           cc=NCC, ct=CT, ts=NTS, sh=PD, f=F)

    store = getattr(nc, store_engine)

    with TileContext(nc) as tc:
        with (
            tc.tile_pool(name="xp", bufs=bufs) as xp,
            tc.tile_pool(name="mp", bufs=bufs) as mp,
            tc.tile_pool(name="cp", bufs=1) as cp,
        ):
            relu_bias = cp.tile([PD, 1], FP32, tag="rbias")
            nc.vector.memset(relu_bias[:], -(MAGIC - LIM))
            tiles = [(ts, cc) for ts in range(NTS) for cc in range(NCC)]
            for ts, cc in tiles * repeat:
                X = xp.tile([PD, CT, F], FP32, tag="x")
                M = mp.tile([PD, NBt, F], FP32, tag="m")
                Q = mp.tile([PD, NBt, F], FP32, tag="q")
                Sc = mp.tile([PD, NBt, F], FP32, tag="s")

                nc.sync.dma_start(X[:], xr[ts, cc])

                if mode == "copy":
                    store.dma_start(orr[ts, cc], X[:])
                    continue

                # block max of |x| over i: AP [p, b, f, i], innermost strided
                nc.vector.tensor_reduce(
                    M[:],
                    X[:].rearrange("p (b i) f -> p b f i", b=NBt, i=blk),
                    axis=mybir.AxisListType.X, op=Alu.max,
                    apply_absolute_value=True,
                )

                Mi = M[:].bitcast(I32)
                Qi = Q[:].bitcast(I32)
                Si = Sc[:].bitcast(I32)
                # V = 0x7F800000 - Ebits   (bitwise complement of exponent field)
                nc.vector.tensor_scalar(
                    Si, Mi, 0x807FFFFF - (1 << 32), -1,
                    op0=Alu.bitwise_or, op1=Alu.bitwise_xor,
                )
                # scale bits = min(V, 0x73000000) + 0x02800000
                nc.vector.tensor_scalar(
                    Si, Si, 0x73000000, 0x02800000,
                    op0=Alu.min, op1=Alu.add,
                )
                # Ebits
                nc.vector.tensor_scalar(
                    Qi, Mi, 0x7F800000, None, op0=Alu.bitwise_and
                )
                # quantum bits = max(Ebits, 0x0C800000) - ((mant-1)<<23)
                nc.vector.tensor_scalar(
                    Qi, Qi, 0x0C800000, EXP_OFF,
                    op0=Alu.max, op1=Alu.subtract,
                )

                X4 = X[:].rearrange("p (b i) f -> p b i f", b=NBt, i=blk)
                Sb = Sc[:].unsqueeze(2).broadcast_to([PD, NBt, blk, F])

                # y = x * scale  (optionally split across POOL and DVE along f)
                if mult_df:
                    k = F - mult_df
                    getattr(nc, mult_engine).tensor_tensor(
                        X4[:, :, :, :k], X4[:, :, :, :k], Sb[:, :, :, :k],
                        op=Alu.mult,
                    )
                    nc.vector.tensor_tensor(
                        X4[:, :, :, k:], X4[:, :, :, k:], Sb[:, :, :, k:],
                        op=Alu.mult,
                    )
                else:
                    getattr(nc, mult_engine).tensor_tensor(X4, X4, Sb, op=Alu.mult)

                X2 = X[:].rearrange("p c f -> p (c f)")
                # t = min(y + MAGIC, MAGIC + lim): RNE round + high clip
                nc.vector.tensor_scalar(
                    X2, X2, MAGIC, MAGIC + LIM, op0=Alu.add, op1=Alu.min
                )

                # a = relu(t - (MAGIC - lim)): low clip
                if relu_engine == "scalar":
                    nc.scalar.activation(
                        X2, X2, mybir.ActivationFunctionType.Relu,
                        bias=relu_bias[:], scale=1.0,
                    )
                else:
                    nc.vector.tensor_scalar(
                        X2, X2, MAGIC - LIM, 0.0,
                        op0=Alu.subtract, op1=Alu.max,
                    )

                # out = (a - lim) * quantum   (per block: STT inputs must be <=3D)
                for b in range(NBt):
                    Qb_b = Q[:, b].unsqueeze(1).broadcast_to([PD, blk, F])
                    nc.vector.scalar_tensor_tensor(
                        X4[:, b], X4[:, b], LIM, Qb_b,
                        op0=Alu.subtract, op1=Alu.mult,
                    )

                store.dma_start(orr[ts, cc], X[:])
    return nc


def _get_nc(mant: int, blk: int, **kw):
    key = (mant, blk, tuple(sorted(kw.items())))
    if key not in _BUILT:
        nc = _build(mant, blk, **kw)
        if not nc.is_finalized():
            nc.finalize()
        _BUILT[key] = nc
    return _BUILT[key]


def kernel(activations, mantissa=7, blk=16, **_ignored):
    from concourse.bass_utils import run_bass_kernel_spmd

    mant = int(np.asarray(mantissa))
    blk = int(np.asarray(blk))
    x = np.asarray(activations, dtype=np.float32)
    assert x.shape == (N, C, D, H, W), x.shape
    assert blk == 16 and C % blk == 0

    nc = _get_nc(mant, blk)
    xf = x.reshape(N, C, S)
    in_maps = [{"x": np.ascontiguousarray(xf[n])} for n in range(N_CORES)]
    res = run_bass_kernel_spmd(nc, in_maps, list(range(N_CORES)))
    outs = [np.asarray(r["o"], dtype=np.float32) for r in res.results]
    return np.stack(outs, axis=0).reshape(N, C, D, H, W)


# revision 19
# speedup vs baseline: 1.1031x; 1.1031x over previous
"""Trainium2 Bass kernel: 3D BFP activation quantization (shared-exponent blocks
of blk=16 contiguous channels along C), data-parallel over batch N across 8
NeuronCores.

kernel(activations[8,64,32,64,64] f32, mantissa=7, blk=16) -> same-shape f32.

Math (exact fp32/int32 bit arithmetic; matches the jnp reference bit-for-bit):
  per block b, spatial s:  M = max_i |x[blk*b+i, s]|
  E  = exponent field of M;  quantum = 2^(E-127-(mant-1));  scale = 1/quantum
  y   = x * scale                                  # exact (pow2), |y| < 2^mant
  t   = min(y + 1.5*2^23, 1.5*2^23 + lim)          # RNE round to int + high clip
  a   = relu(t - (1.5*2^23 - lim))                 # low clip; a = clipped + lim
  out = (a - lim) * quantum                        # exact

Bit manipulation (int32, avoiding signed overflow and bitwise/arith mixing):
  V  = (Mbits | 0x807FFFFF) ^ -1   = 0x7F800000 - Ebits          (pure bitwise)
  Sbits = min(V, 0x73000000) + 0x02800000                        (scale, clamped
          so an all-zero block yields a finite scale; exact for any E >= 25)
  Ebits = Mbits & 0x7F800000
  Qbits = max(Ebits, 0x0C800000) - ((mant-1)<<23)                (quantum)

Layout: partition = 128 spatial positions, free = (C=64 channels, F spatial).
One 3D DMA per tile; block reduction over i done with a strided access pattern.
"""

import os
import sys

for _p in ("/opt/trn_rl_repo", "/root/.axon_site/_ro/trn_rl_repo"):
    if os.path.isdir(_p) and _p not in sys.path:
        sys.path.insert(0, _p)

import numpy as np

# ---- hardcoded problem geometry ----
N, C, D, H, W = 8, 64, 32, 64, 64
S = D * H * W                 # 131072 spatial per (n, c)
N_CORES = 8
PD = 128                      # SBUF partitions (spatial)

_BUILT = {}


def _build(mant: int, blk: int, F: int = 512, CT: int = 16, bufs: int = 3,
           mult_engine: str = "gpsimd", relu_engine: str = "scalar",
           store_engine: str = "sync", mult_df: int = 0,
           C: int = C, S: int = S, mode: str = "full", repeat: int = 1):
    import concourse.bass as bass
    import concourse.bacc as bacc
    import concourse.mybir as mybir
    from concourse.tile import TileContext

    FP32 = mybir.dt.float32
    I32 = mybir.dt.int32
    Alu = mybir.AluOpType

    NBt = CT // blk           # channel blocks per tile
    NCC = C // CT             # channel chunks
    NTS = S // (PD * F)       # spatial chunks
    assert S % (PD * F) == 0 and C % CT == 0 and CT % blk == 0

    LIM = float(2 ** mant - 1)
    MAGIC = 1.5 * 2.0 ** 23
    EXP_OFF = (mant - 1) << 23

    nc = bacc.Bacc("TRN2", target_bir_lowering=False)
    x_d = nc.dram_tensor("x", [C, S], FP32, kind="ExternalInput")
    o_d = nc.dram_tensor("o", [C, S], FP32, kind="ExternalOutput")

    xr = x_d[:].rearrange("(cc ct) (ts sh f) -> ts cc sh ct f",
                          cc=NCC, ct=CT, ts=NTS, sh=PD, f=F)
    orr = o_d[:].rearrange("(cc ct) (ts sh f) -> ts cc sh ct f",
                           cc=NCC, ct=CT, ts=NTS, sh=PD, f=F)

    store = getattr(nc, store_engine)

    with TileContext(nc) as tc:
        with (
            tc.tile_pool(name="xp", bufs=bufs) as xp,
            tc.tile_pool(name="mp", bufs=bufs) as mp,
            tc.tile_pool(name="cp", bufs=1) as cp,
        ):
            relu_bias = cp.tile([PD, 1], FP32, tag="rbias")
            nc.vector.memset(relu_bias[:], -(MAGIC - LIM))
            tl = [(ts, cc) for ts in range(NTS) for cc in range(NCC)] * repeat

            # Loads are issued `bufs` tiles ahead of stores so the SP
            # sequencer (which blocks at each store's semaphore wait) never
            # gates the next load behind the current tile's compute.
            staged = {}

            def issue_load(i):
                ts_, cc_ = tl[i]
                Xl = xp.tile([PD, CT, F], FP32, tag="x")
                Ml = mp.tile([PD, NBt, F], FP32, tag="m")
                Ql = mp.tile([PD, NBt, F], FP32, tag="q")
                Sl = mp.tile([PD, NBt, F], FP32, tag="s")
                nc.sync.dma_start(Xl[:], xr[ts_, cc_])
                staged[i] = (Xl, Ml, Ql, Sl)

            for i in range(min(bufs, len(tl))):
                issue_load(i)

            for i, (ts, cc) in enumerate(tl):
                X, M, Q, Sc = staged.pop(i)

                if mode == "copy":
                    store.dma_start(orr[ts, cc], X[:])
                    if i + bufs < len(tl):
                        issue_load(i + bufs)
                    continue

                # block max of |x| over i: AP [p, b, f, i], innermost strided
                nc.vector.tensor_reduce(
                    M[:],
                    X[:].rearrange("p (b i) f -> p b f i", b=NBt, i=blk),
                    axis=mybir.AxisListType.X, op=Alu.max,
                    apply_absolute_value=True,
                )

                Mi = M[:].bitcast(I32)
                Qi = Q[:].bitcast(I32)
                Si = Sc[:].bitcast(I32)
                # V = 0x7F800000 - Ebits   (bitwise complement of exponent field)
                nc.vector.tensor_scalar(
                    Si, Mi, 0x807FFFFF - (1 << 32), -1,
                    op0=Alu.bitwise_or, op1=Alu.bitwise_xor,
                )
                # scale bits = min(V, 0x73000000) + 0x02800000
                nc.vector.tensor_scalar(
                    Si, Si, 0x73000000, 0x02800000,
                    op0=Alu.min, op1=Alu.add,
                )
                # Ebits
                nc.vector.tensor_scalar(
                    Qi, Mi, 0x7F800000, None, op0=Alu.bitwise_and
                )
                # quantum bits = max(Ebits, 0x0C800000) - ((mant-1)<<23)
                nc.vector.tensor_scalar(
                    Qi, Qi, 0x0C800000, EXP_OFF,
                    op0=Alu.max, op1=Alu.subtract,
                )

                X4 = X[:].rearrange("p (b i) f -> p b i f", b=NBt, i=blk)
                Sb = Sc[:].unsqueeze(2).broadcast_to([PD, NBt, blk, F])

                # y = x * scale  (optionally split across POOL and DVE along f)
                if mult_df:
                    k = F - mult_df
                    getattr(nc, mult_engine).tensor_tensor(
                        X4[:, :, :, :k], X4[:, :, :, :k], Sb[:, :, :, :k],
                        op=Alu.mult,
                    )
                    nc.vector.tensor_tensor(
                        X4[:, :, :, k:], X4[:, :, :, k:], Sb[:, :, :, k:],
                        op=Alu.mult,
                    )
                else:
                    getattr(nc, mult_engine).tensor_tensor(X4, X4, Sb, op=Alu.mult)

                X2 = X[:].rearrange("p c f -> p (c f)")
                # t = min(y + MAGIC, MAGIC + lim): RNE round + high clip
                nc.vector.tensor_scalar(
                    X2, X2, MAGIC, MAGIC + LIM, op0=Alu.add, op1=Alu.min
                )

                # a = relu(t - (MAGIC - lim)): low clip
                if relu_engine == "scalar":
                    nc.scalar.activation(
                        X2, X2, mybir.ActivationFunctionType.Relu,
                        bias=relu_bias[:], scale=1.0,
                    )
                else:
                    nc.vector.tensor_scalar(
                        X2, X2, MAGIC - LIM, 0.0,
                        op0=Alu.subtract, op1=Alu.max,
                    )

                # out = (a - lim) * quantum   (per block: STT inputs must be <=3D)
                for b in range(NBt):
                    Qb_b = Q[:, b].unsqueeze(1).broadcast_to([PD, blk, F])
                    nc.vector.scalar_tensor_tensor(
                        X4[:, b], X4[:, b], LIM, Qb_b,
                        op0=Alu.subtract, op1=Alu.mult,
                    )

                store.dma_start(orr[ts, cc], X[:])
                if i + bufs < len(tl):
                    issue_load(i + bufs)
    return nc


def _get_nc(mant: int, blk: int, **kw):
    key = (mant, blk, tuple(sorted(kw.items())))
    if key not in _BUILT:
        nc = _build(mant, blk, **kw)
        if not nc.is_finalized():
            nc.finalize()
        _BUILT[key] = nc
    return _BUILT[key]


def kernel(activations, mantissa=7, blk=16, **_ignored):
    from concourse.bass_utils import run_bass_kernel_spmd

    mant = int(np.asarray(mantissa))
    blk = int(np.asarray(blk))
    x = np.asarray(activations, dtype=np.float32)
    assert x.shape == (N, C, D, H, W), x.shape
    assert blk == 16 and C % blk == 0

    nc = _get_nc(mant, blk)
    xf = x.reshape(N, C, S)
    in_maps = [{"x": np.ascontiguousarray(xf[n])} for n in range(N_CORES)]
    res = run_bass_kernel_spmd(nc, in_maps, list(range(N_CORES)))
    outs = [np.asarray(r["o"], dtype=np.float32) for r in res.results]
    return np.stack(outs, axis=0).reshape(N, C, D, H, W)
